# revision 1
# baseline (speedup 1.0000x reference)
"""Trainium2 Bass kernel for nn_Attention_org_45758581571643.

Reference computation (per batch b):
  x = emb[b] viewed as [S=T*N, C] (token-major)
  per head h: Q/K/V = x @ W{q,k,v}[h].T ; scores = Q K^T / sqrt(S)
  InstanceNorm over each [S,S] map, softmax over keys, ctx = probs @ V
  out = mean_h(ctx) @ Wo.T, reshaped to [B, T, C, N]

Sharding: 16 (batch, head) pairs over 8 cores -> core c handles batch c//2,
heads {2*(c%2), 2*(c%2)+1}. Head-mean and the Wo projection are linear, so each
core applies Wo to its own two-head partial sum and the host adds core pairs.

On-device layout is fully transposed: x/Q/K live as [C, S] (channel on
partitions), scores as [t, s] (keys on partitions). Softmax runs over the
partition axis: denominators via ones-matmuls on the PE, stats via
ACT-copy-with-accum + DVE square-reduce + gpsimd partition_all_reduce.
probs @ V then needs no transposes at all. The 1/sqrt(S) score scaling is
skipped -- instance norm is invariant to it. S is zero-padded to 1664 = 13*128;
padded key/value rows are exactly zero so sums and matmuls stay exact, and the
padded rows are excluded from softmax denominators by a K=32 tail matmul.
"""

import os

# Recover gracefully if a previous run left a NeuronCore wedged; must be set
# before the runtime initializes.
os.environ.setdefault("NEURON_RT_RESET_CORES", "1")

import numpy as np
from contextlib import ExitStack

B, T, C, N, H = 4, 8, 256, 196, 4
S = T * N          # 1568
SP = 1664          # 13 * 128 (padded key/seq length)
NT = SP // 128     # 13 t-tiles
SCW = 392          # s-chunk width (4 * 392 = 1568)
NSC = S // SCW     # 4
PAD_REAL = S - (NT - 1) * 128  # 32 real rows in the last t-tile
EPS = 1e-5
COUNT = float(S) * float(S)

_CACHE = {}


def _build_nc(reps=1):
    import concourse.bass as bass
    import concourse.tile as tile
    from concourse import bacc, bass_isa, mybir

    f32 = mybir.dt.float32
    f32r = mybir.dt.float32r
    AF = mybir.ActivationFunctionType
    ALU = mybir.AluOpType

    nc = bacc.Bacc("TRN2", target_bir_lowering=False, debug=False)

    xt_d = nc.dram_tensor("xt", [C, SP], f32r, kind="ExternalInput").ap()
    wg_d = nc.dram_tensor("wg", [2, C, C], f32r, kind="ExternalInput").ap()
    wvo_d = nc.dram_tensor("wvo", [2, C, C], f32r, kind="ExternalInput").ap()
    ot_d = nc.dram_tensor("ot", [C, S], f32, kind="ExternalOutput").ap()

    def r(ap):
        return ap

    def v32(ap):
        return ap.bitcast(f32)

    with tile.TileContext(nc) as tc, ExitStack() as ctx:
        xw = ctx.enter_context(tc.tile_pool(name="xw", bufs=1))
        qk = ctx.enter_context(tc.tile_pool(name="qk", bufs=1))
        vp = ctx.enter_context(tc.tile_pool(name="vp", bufs=1))
        sc = ctx.enter_context(tc.tile_pool(name="sc", bufs=1))
        cx = ctx.enter_context(tc.tile_pool(name="cx", bufs=1))
        sm = ctx.enter_context(tc.tile_pool(name="sm", bufs=4))
        scr = ctx.enter_context(tc.tile_pool(name="scr", bufs=2))
        pmm = ctx.enter_context(tc.tile_pool(name="pmm", bufs=3, space="PSUM"))
        pcx = ctx.enter_context(tc.tile_pool(name="pcx", bufs=3, space="PSUM"))
        pcs = ctx.enter_context(tc.tile_pool(name="pcs", bufs=2, space="PSUM"))

        # ---- load inputs (weights first; xt chunk-major on two queues) ----
        wsb = {}
        for nm, d, eng in (("wg", wg_d, nc.scalar), ("wvo", wvo_d, nc.scalar)):
            for h in range(2):
                for cti in range(2):
                    t = xw.tile([128, C], f32r, tag=f"{nm}{h}{cti}", name=f"{nm}{h}{cti}")
                    eng.dma_start(t[:], d[h, cti * 128:(cti + 1) * 128, :])
                    wsb[nm, h, cti] = t
        xt = [xw.tile([128, SP], f32r, tag=f"xt{i}", name=f"xt{i}") for i in range(2)]
        for kci in range(4):
            kl = slice(kci * 416, (kci + 1) * 416)
            for cti in range(2):
                eng = nc.sync if cti == 0 else nc.gpsimd
                eng.dma_start(xt[cti][:, kl],
                              xt_d[cti * 128:(cti + 1) * 128, kl])

        fourf = xw.tile([128, 1], f32, tag="fourf")
        nc.vector.memset(fourf, float(H))
        four = xw.tile([128, 1], f32r, tag="four")
        nc.vector.tensor_copy(four[:], fourf[:])
        epsb = xw.tile([128, 1], f32, tag="epsb")
        nc.vector.memset(epsb, EPS)

        def body():
            # dti -> [128, S]: output^T accumulated over this core's heads
            ctxs = {}
            for dti in range(2):
                ctxs[dti] = cx.tile([128, S], f32, tag=f"ctx{dti}", name=f"ctx{dti}", bufs=1)

            for h in range(2):
                # ---- G[c,s] = (Wq^T Wk)^T x^T : scores = x @ G ; V' = x @ (Wv^T Wo^T)
                gt = [qk.tile([128, S], f32r, tag=f"gt{i}", name=f"gt{i}", bufs=2) for i in range(2)]
                for dti in range(2):
                    for sci in range(NSC):
                        sl = slice(sci * SCW, (sci + 1) * SCW)
                        ps = pmm.tile([128, SCW], f32, tag="ps", name="ps")
                        for cti in range(2):
                            nc.tensor.matmul(
                                ps[:], r(wsb["wg", h, cti][:, dti * 128:(dti + 1) * 128]),
                                r(xt[cti][:, sl]), start=(cti == 0), stop=(cti == 1))
                        nc.vector.tensor_copy(gt[dti][:, sl], ps[:])
                v = [vp.tile([128, C], f32r, tag=f"v{i}", name=f"v{i}", bufs=2) for i in range(NT)]
                for ti in range(NT):
                    tsl = slice(ti * 128, (ti + 1) * 128)
                    ps = pmm.tile([128, C], f32, tag="ps", name="ps")
                    for cti in range(2):
                        nc.tensor.matmul(
                            ps[:], r(xt[cti][:, tsl]), r(wsb["wvo", h, cti][:]),
                            start=(cti == 0), stop=(cti == 1))
                    nc.vector.tensor_copy(v[ti][:], ps[:])

                # ---- scores^T [t, s] + per-tile stats (bn_stats -> sums) ----
                sums = sm.tile([128, NT], f32, tag="sums", name="sums")
                sumsq = sm.tile([128, NT], f32, tag="sumsq", name="sumsq")
                st = [sc.tile([128, S], f32r, tag=f"st{i}", name=f"st{i}",
                              bufs=(2 if i < 4 else 1)) for i in range(NT)]
                for ti in range(NT):
                    tsl = slice(ti * 128, (ti + 1) * 128)
                    bst = sm.tile([128, NSC, 6], f32, tag="bst", name="bst", bufs=3)
                    for sci in range(NSC):
                        sl = slice(sci * SCW, (sci + 1) * SCW)
                        ps = pmm.tile([128, SCW], f32, tag="ps", name="ps")
                        for cti in range(2):
                            nc.tensor.matmul(
                                ps[:], r(xt[cti][:, tsl]), r(gt[cti][:, sl]),
                                start=(cti == 0), stop=(cti == 1))
                        nc.scalar.activation(out=st[ti][:, sl], in_=ps[:], func=AF.Copy)
                        nc.vector.bn_stats(out=bst[:, sci, :], in_=v32(st[ti][:, sl]))
                    mv = sm.tile([128, 2], f32, tag="mv", name="mv", bufs=3)
                    nc.vector.bn_aggr(out=mv[:], in_=bst[:])
                    # per-row sum = mean * S ; sumsq = (var + mean^2) * S
                    nc.vector.tensor_scalar_mul(sums[:, ti:ti + 1], mv[:, 0:1], float(S))
                    t2 = sm.tile([128, 1], f32, tag="t2s", name="t2s", bufs=3)
                    nc.vector.scalar_tensor_tensor(
                        out=t2[:], in0=mv[:, 0:1], scalar=mv[:, 0:1], in1=mv[:, 1:2],
                        op0=ALU.mult, op1=ALU.add)
                    nc.vector.tensor_scalar_mul(sumsq[:, ti:ti + 1], t2[:], float(S))

                # ---- instance-norm scalars (replicated across partitions) ----
                st2 = sm.tile([128, 2], f32, tag="st2", name="st2")
                nc.vector.tensor_reduce(st2[:, 0:1], sums[:], axis=mybir.AxisListType.X,
                                        op=ALU.add)
                nc.vector.tensor_reduce(st2[:, 1:2], sumsq[:], axis=mybir.AxisListType.X,
                                        op=ALU.add)
                red = sm.tile([128, 2], f32, tag="red", name="red")
                nc.gpsimd.partition_all_reduce(red[:], st2[:], channels=128,
                                               reduce_op=bass_isa.ReduceOp.add)
                me = sm.tile([128, 2], f32, tag="me", name="me", bufs=3)
                nc.vector.tensor_scalar_mul(me[:], red[:], 1.0 / COUNT)
                mean = me[:, 0:1]
                mm2 = sm.tile([128, 1], f32, tag="mm2", name="mm2", bufs=3)
                nc.vector.tensor_mul(mm2[:], mean, mean)
                ve = sm.tile([128, 1], f32, tag="ve", name="ve", bufs=3)
                nc.vector.scalar_tensor_tensor(
                    out=ve[:], in0=me[:, 1:2], scalar=EPS, in1=mm2[:],
                    op0=ALU.add, op1=ALU.subtract)
                # rstd = 1/sqrt(ve) on DVE (magic + 2 Newton) -- keeps Sqrt off
                # ACT so its table set stays Copy/Exp (no mid-chain table loads)
                i32 = mybir.dt.int32
                half = sm.tile([128, 1], f32, tag="half", name="half", bufs=3)
                nc.vector.tensor_scalar_mul(half[:], ve[:], 0.5)
                yi = sm.tile([128, 1], i32, tag="yi", name="yi", bufs=3)
                nc.vector.tensor_scalar(
                    out=yi[:], in0=ve[:].bitcast(i32), scalar1=1, scalar2=None,
                    op0=ALU.arith_shift_right)
                nc.vector.tensor_scalar(
                    out=yi[:], in0=yi[:], scalar1=-1, scalar2=0x5F3759DF,
                    op0=ALU.mult, op1=ALU.add)
                rstd = sm.tile([128, 1], f32, tag="rstd", name="rstd")
                t4 = sm.tile([128, 1], f32, tag="t4", name="t4", bufs=3)
                y = yi[:].bitcast(f32)
                for _nw in range(2):
                    nc.vector.tensor_mul(t4[:], y, y)
                    nc.vector.tensor_mul(t4[:], t4[:], half[:])
                    nc.vector.tensor_scalar(
                        out=t4[:], in0=t4[:], scalar1=-1.0, scalar2=1.5,
                        op0=ALU.mult, op1=ALU.add)
                    nc.vector.tensor_mul(rstd[:], y, t4[:])
                    y = rstd[:]
                nbias = sm.tile([128, 1], f32, tag="nbias", name="nbias")
                nc.vector.scalar_tensor_tensor(
                    out=nbias[:], in0=mean, scalar=-1.0, in1=rstd[:],
                    op0=ALU.mult, op1=ALU.mult)

                # ---- p = exp(rstd * scores + nbias), in place ----
                for ti in range(NT):
                    nc.scalar.activation(out=st[ti][:], in_=st[ti][:], func=AF.Exp,
                                         bias=nbias[:], scale=rstd[:])

                # ---- softmax denominators (x H): per-chunk recip + broadcast ----
                den = sm.tile([1, S], f32, tag="den", name="den", bufs=1)
                recipb = scr.tile([128, S], f32, tag="recipb", name="recipb", bufs=1)
                for sci in range(NSC):
                    sl = slice(sci * SCW, (sci + 1) * SCW)
                    pd = pcs.tile([1, SCW], f32, tag="pd", name="pd")
                    for ti in range(NT):
                        kk = 128 if ti < NT - 1 else PAD_REAL
                        nc.tensor.matmul(
                            pd[:], r(four[0:kk, :]), r(st[ti][0:kk, sl]),
                            start=(ti == 0), stop=(ti == NT - 1))
                    nc.vector.reciprocal(den[0:1, sl], pd[:])
                    nc.gpsimd.partition_broadcast(recipb[:, sl], den[0:1, sl])

                # ---- ctx^T[d, s] = V^T p, scaled by 1/(H * denom) ----
                for dti in range(2):
                    dsl = slice(dti * 128, (dti + 1) * 128)
                    for sci in range(NSC):
                        sl = slice(sci * SCW, (sci + 1) * SCW)
                        ps = pcx.tile([128, SCW], f32, tag="psx", name="psx")
                        for ti in range(NT):
                            nc.tensor.matmul(ps[:], r(v[ti][:, dsl]), r(st[ti][:, sl]),
                                             start=(ti == 0), stop=(ti == NT - 1))
                        if h == 0:
                            nc.vector.tensor_mul(ctxs[dti][:, sl], ps[:], recipb[:, sl])
                        else:
                            t3 = scr.tile([128, SCW], f32, tag="t2", name="t3")
                            nc.vector.tensor_mul(t3[:], ps[:], recipb[:, sl])
                            nc.vector.tensor_add(ctxs[dti][:, sl], ctxs[dti][:, sl], t3[:])

            # ---- write out: ot[e, s] = sum_h (V'^T p)/(H*denom) ----
            for eti in range(2):
                esl = slice(eti * 128, (eti + 1) * 128)
                nc.sync.dma_start(ot_d[esl, :], ctxs[eti][:])


        for _ in range(reps):
            body()

    nc.finalize()
    return nc


def _get_nc(reps=1):
    key = ("nc", reps)
    if key not in _CACHE:
        _CACHE[key] = _build_nc(reps)
    return _CACHE[key]


def make_in_maps(emb, Wq, Wk, Wv, Wo):
    emb = np.ascontiguousarray(emb, dtype=np.float32)
    Wq = np.asarray(Wq, np.float64)
    Wk = np.asarray(Wk, np.float64)
    Wv = np.asarray(Wv, np.float64)
    Wo = np.asarray(Wo, np.float64)
    # wg[h] = Wq[h]^T @ Wk[h]  (scores = x wg^T x^T per head, see kernel docstring)
    wg = np.einsum("hdc,hde->hce", Wq, Wk).astype(np.float32)
    # wvo[h] = Wv[h]^T @ Wo^T  (folds the output projection into V)
    wvo = np.einsum("hdc,ed->hce", Wv, Wo).astype(np.float32)
    in_maps = []
    for core in range(8):
        b, g = core // 2, core % 2
        xt = np.zeros((C, SP), np.float32)
        xt[:, :S] = emb[b].transpose(1, 0, 2).reshape(C, S)
        hs = [2 * g, 2 * g + 1]
        in_maps.append({
            "xt": xt,
            "wg": np.ascontiguousarray(wg[hs]),
            "wvo": np.ascontiguousarray(wvo[hs]),
        })
    return in_maps


def gather_out(results):
    out = np.empty((B, S, C), np.float32)
    for b in range(B):
        out[b] = (results[2 * b]["ot"] + results[2 * b + 1]["ot"]).T
    return out.reshape(B, T, C, N)


def _get_runner():
    """Cached PJRT executable: run_bass_kernel_spmd re-jits per call, which
    costs seconds of XLA compile on every invocation; build the sharded
    callable once and reuse it."""
    if "runner" in _CACHE:
        return _CACHE["runner"]
    import jax
    from jax.sharding import Mesh, PartitionSpec, NamedSharding
    from jax.experimental.shard_map import shard_map
    from concourse import mybir
    from concourse.bass2jax import (_bass_exec_p, install_neuronx_cc_hook,
                                    partition_id_tensor)

    install_neuronx_cc_hook()
    nc = _get_nc()
    in_names, out_names, out_avals, zero_shapes = [], [], [], []
    partition_name = nc.partition_id_tensor.name if nc.partition_id_tensor else None
    for alloc in nc.m.functions[0].allocations:
        if not isinstance(alloc, mybir.MemoryLocationSet):
            continue
        name = alloc.memorylocations[0].name
        if alloc.kind == "ExternalInput":
            if name != partition_name:
                in_names.append(name)
        elif alloc.kind == "ExternalOutput":
            shape = tuple(alloc.tensor_shape)
            dtype = mybir.dt.np(alloc.dtype)
            out_names.append(name)
            out_avals.append(jax.core.ShapedArray(shape, dtype))
            zero_shapes.append((shape, dtype))
    n_params = len(in_names)
    all_in = list(in_names) + list(out_names)
    if partition_name is not None:
        all_in.append(partition_name)

    def _body(*args):
        operands = list(args)
        if partition_name is not None:
            operands.append(partition_id_tensor())
        return tuple(_bass_exec_p.bind(
            *operands, out_avals=tuple(out_avals), in_names=tuple(all_in),
            out_names=tuple(out_names), lowering_input_output_aliases=(),
            sim_require_finite=True, sim_require_nnan=True, nc=nc))

    n_cores = 8
    mesh = Mesh(np.asarray(jax.devices()[:n_cores]), ("core",))
    sharded = jax.jit(
        shard_map(_body, mesh=mesh,
                  in_specs=(PartitionSpec("core"),) * (n_params + len(out_names)),
                  out_specs=(PartitionSpec("core"),) * len(out_names),
                  check_rep=False),
        keep_unused=True)

    def run(in_maps):
        per_core = [[np.asarray(m[nm]) for nm in in_names] for m in in_maps]
        concat_in = [np.concatenate([per_core[c][i] for c in range(n_cores)], axis=0)
                     for i in range(n_params)]
        concat_zeros = [np.zeros((n_cores * s[0], *s[1:]), d)
                        for (s, d) in zero_shapes]
        outs = sharded(*concat_in, *concat_zeros)
        return [{out_names[i]: np.asarray(outs[i]).reshape(
                     n_cores, *out_avals[i].shape)[c]
                 for i in range(len(out_names))} for c in range(n_cores)]

    _CACHE["runner"] = run
    return run


def kernel(emb, Wq, Wk, Wv, Wo):
    in_maps = make_in_maps(emb, Wq, Wk, Wv, Wo)
    try:
        return gather_out(_get_runner()(in_maps))
    except Exception:
        from concourse.bass_utils import run_bass_kernel_spmd
        nc = _get_nc()
        res = run_bass_kernel_spmd(nc, in_maps, list(range(8)))
        return gather_out(res.results)



# revision 17
# speedup vs baseline: 1.7302x; 1.7302x over previous
"""Trainium2 Bass kernel for nn_Attention_org_45758581571643.

Reference (per batch b):
  x = emb[b] viewed as [S=T*N, C] (token-major)
  per head h: scores[s,t] = x_s (Wq^T Wk) x_t ; InstanceNorm over [S,S],
  softmax over t, ctx = probs @ V ; out = mean_h(ctx) @ Wo^T.

Sharding: 16 (batch, head) pairs over 8 cores -> core c handles batch c//2,
heads {2*(c%2), 2*(c%2)+1}; host adds the two half-batch outputs.

Design (vs the f32r two-pass baseline):
  * All PE matmuls in bf16 (1 cycle/column, half the DMA/SBUF of f32r).
  * InstanceNorm stats computed ANALYTICALLY before the scores matmul:
    sum(scores) = u^T Wg u and sum(scores^2) = tr(Wg M Wg^T M) with
    M = x^T x (Gram matrix, shared by both heads, symmetric -- so both
    Wg M and (M Wg) come from the same two SBUF operands) and u = x^T 1
    (the ones-column of the M matmul). Softmax is shift-invariant, so
    mean enters only through var = E[s^2] - mean^2.
  * exp fused into the single PSUM->SBUF drain of the scores matmul
    (ACT, scale=rstd) -- no separate copy pass, no bn_stats.
  * ctx computed transposed: ctxT[s,d] = sum_t p[t,s] V'[t,d], with a
    ones-indicator column appended to V' so the softmax denominator
    materializes as PSUM column 256 of the same matmul chain. The
    1/denominator is then a per-partition scalar: one DVE op per s-tile.
  * Wv/Wo and the mean over heads folded into V' on the host.
Engines: PE matmuls; ACT exp + G drains; DVE stats, V' drains,
reciprocals, ctx scaling; Pool only partition_all_reduce + one DMA queue.
"""

import os

os.environ.setdefault("NEURON_RT_RESET_CORES", "1")

import numpy as np
from contextlib import ExitStack

B, T, C, N, H = 4, 8, 256, 196, 4
S = T * N            # 1568
SP = 1664            # 13 * 128 padded t length
NT = SP // 128       # 13 t-tiles
NS = (S + 127) // 128  # 13 s-tiles (last has 32 rows)
LAST_T = S - (NT - 1) * 128   # 32 real rows in t-tile 12
XSW = 258            # xs tile row stride (even -> 4-byte aligned bf16 slices)
EPS = 1e-5
CNT = float(S) * float(S)

_CACHE = {}


def _build_nc(reps=1):
    import concourse.bass as bass
    import concourse.tile as tile
    from concourse import bacc, bass_isa, mybir

    f32 = mybir.dt.float32
    bf16 = mybir.dt.bfloat16
    i32 = mybir.dt.int32
    AF = mybir.ActivationFunctionType
    ALU = mybir.AluOpType

    nc = bacc.Bacc("TRN2", target_bir_lowering=False, debug=False)

    xtb_d = nc.dram_tensor("xtb", [128, 2 * SP], bf16, kind="ExternalInput").ap()
    xsb_d = nc.dram_tensor("xsb", [128, NT * XSW], bf16, kind="ExternalInput").ap()
    wgb_d = nc.dram_tensor("wgb", [2, 128, 2 * 256], bf16, kind="ExternalInput").ap()
    wgbt_d = nc.dram_tensor("wgbt", [2, 128, 2 * 256], bf16, kind="ExternalInput").ap()
    wvob_d = nc.dram_tensor("wvob", [2, 128, 2 * 256], bf16, kind="ExternalInput").ap()
    ot_d = nc.dram_tensor("ot", [S, C], f32, kind="ExternalOutput").ap()

    # scores/G psum chunking: one PSUM bank (512 f32) per chunk
    CH = [(0, 512), (512, 1024), (1024, 1536), (1536, 1568)]

    with tile.TileContext(nc) as tc, ExitStack() as ctx:
        xw = ctx.enter_context(tc.tile_pool(name="xw", bufs=1))
        stp = ctx.enter_context(tc.tile_pool(name="stp", bufs=1))
        gv = ctx.enter_context(tc.tile_pool(name="gv", bufs=1))
        cxo = ctx.enter_context(tc.tile_pool(name="cxo", bufs=1))
        sm = ctx.enter_context(tc.tile_pool(name="sm", bufs=1))
        psc = ctx.enter_context(tc.tile_pool(name="psc", bufs=6, space="PSUM"))
        pcx = ctx.enter_context(tc.tile_pool(name="pcx", bufs=2, space="PSUM"))

        # ---- input DMAs (split across queues) ----
        xsb = xw.tile([128, NT * XSW], bf16, tag="xsb", name="xsb")
        nc.sync.dma_start(xsb[:], xsb_d[:, :])
        xtb = xw.tile([128, 2 * SP], bf16, tag="xtb", name="xtb")
        nc.gpsimd.dma_start(xtb[:], xtb_d[:, :])
        wgb, wgbt, wvob = {}, {}, {}
        for h in range(2):
            wgb[h] = xw.tile([128, 2 * 256], bf16, tag=f"wg{h}", name=f"wg{h}")
            nc.scalar.dma_start(wgb[h][:], wgb_d[h, :, :])
            wgbt[h] = xw.tile([128, 2 * 256], bf16, tag=f"wgt{h}", name=f"wgt{h}")
            nc.scalar.dma_start(wgbt[h][:], wgbt_d[h, :, :])
            wvob[h] = xw.tile([128, 2 * 256], bf16, tag=f"wv{h}", name=f"wv{h}")
            nc.scalar.dma_start(wvob[h][:], wvob_d[h, :, :])

        def rsqrt_dve(ve, tagp):
            """rstd = 1/sqrt(ve), magic + 2 Newton steps on DVE ([128,1])."""
            half = sm.tile([128, 1], f32, tag=f"{tagp}half", name=f"{tagp}half")
            nc.vector.tensor_scalar_mul(half[:], ve[:], 0.5)
            yi = sm.tile([128, 1], i32, tag=f"{tagp}yi", name=f"{tagp}yi")
            nc.vector.tensor_scalar(
                out=yi[:], in0=ve[:].bitcast(i32), scalar1=1, scalar2=None,
                op0=ALU.arith_shift_right)
            nc.vector.tensor_scalar(
                out=yi[:], in0=yi[:], scalar1=-1, scalar2=0x5F3759DF,
                op0=ALU.mult, op1=ALU.add)
            rstd = sm.tile([128, 1], f32, tag=f"{tagp}rstd", name=f"{tagp}rstd")
            t4 = sm.tile([128, 1], f32, tag=f"{tagp}t4", name=f"{tagp}t4")
            y = yi[:].bitcast(f32)
            for _ in range(2):
                nc.vector.tensor_mul(t4[:], y, y)
                nc.vector.tensor_mul(t4[:], t4[:], half[:])
                nc.vector.tensor_scalar(
                    out=t4[:], in0=t4[:], scalar1=-1.0, scalar2=1.5,
                    op0=ALU.mult, op1=ALU.add)
                nc.vector.tensor_mul(rstd[:], y, t4[:])
                y = rstd[:]
            return rstd

        DBG = int(os.environ.get("KDBG", "0"))

        def body():
            if DBG == 8:
                z = sm.tile([128, 1], f32, tag="z8", name="z8")
                nc.vector.memset(z[:], 3.25)
                nc.sync.dma_start(ot_d[0:128, 0:1], z[:])
                return
            if DBG == 9:
                z = sm.tile([128, 256], f32, tag="z9", name="z9")
                nc.vector.memset(z[:], 1.5)
                zb = sm.tile([128, 256], bf16, tag="z9b", name="z9b")
                nc.vector.memset(zb[:], 2.0)
                part = sm.tile([128, 1], f32, tag="pt9", name="pt9")
                prod = sm.tile([128, 256], f32, tag="prod9", name="prod9")
                nc.vector.tensor_tensor_reduce(
                    out=prod[:], in0=z[:], in1=zb[:],
                    scale=1.0, scalar=0.0, op0=ALU.mult, op1=ALU.add,
                    accum_out=part[:])
                mo = sm.tile([128, 256], f32, tag="mo", name="mo")
                nc.vector.tensor_copy(mo[:], prod[:])
                nc.sync.dma_start(ot_d[0:128, 0:256], mo[:])
                return
            if DBG in (4, 5, 6, 7):
                Mb = []
                for half in range(2):
                    mm = pcx.tile([128, 512], f32, tag="cx", name="mx")
                    for ti in range(NT):
                        o = ti * XSW
                        nc.tensor.matmul(
                            mm[:, 0:257], xsb[:, o + half * 128:o + half * 128 + 128],
                            xsb[:, o:o + 257], start=(ti == 0), stop=(ti == NT - 1))
                    mb = sm.tile([128, 257], bf16, tag=f"mb{half}", name=f"mb{half}")
                    nc.vector.tensor_copy(mb[:], mm[:, 0:257])
                    Mb.append(mb)
                if DBG == 4:
                    mo = sm.tile([128, 256], f32, tag="mo", name="mo")
                    nc.vector.tensor_copy(mo[:], Mb[0][:, 0:256])
                    nc.sync.dma_start(ot_d[0:128, 0:256], mo[:])
                    return
                h, dti = 0, 0
                dps = pcx.tile([128, 512], f32, tag="cx", name="dps")
                for k in range(2):
                    nc.tensor.matmul(
                        dps[:, 0:256], wgb[h][:, k * 256 + dti * 128:
                                              k * 256 + dti * 128 + 128],
                        Mb[k][:, 0:256], start=(k == 0), stop=(k == 1))
                db = sm.tile([128, 256], bf16, tag="db", name="db", bufs=2)
                nc.vector.tensor_copy(db[:], dps[:, 0:256])
                fps = pcx.tile([128, 512], f32, tag="cx", name="fps")
                for k in range(2):
                    nc.tensor.matmul(
                        fps[:, 0:256], Mb[k][:, dti * 128:dti * 128 + 128],
                        wgb[h][:, k * 256:k * 256 + 256],
                        start=(k == 0), stop=(k == 1))
                if DBG == 5:
                    mo = sm.tile([128, 256], f32, tag="mo", name="mo")
                    nc.vector.tensor_copy(mo[:], fps[:, 0:256])
                    nc.sync.dma_start(ot_d[0:128, 0:256], mo[:])
                    return
                part = sm.tile([128, 1], f32, tag="pt00", name="pt00")
                nc.vector.tensor_tensor_reduce(
                    out=sm.tile([128, 256], f32, tag="prod", name="prod", bufs=2)[:],
                    in0=fps[:, 0:256], in1=db[:],
                    scale=1.0, scalar=0.0, op0=ALU.mult, op1=ALU.add,
                    accum_out=part[:])
                if DBG == 6:
                    nc.sync.dma_start(ot_d[0:128, 0:1], part[:])
                    return
                wps = pcx.tile([128, 512], f32, tag="cx", name="wps")
                for k in range(2):
                    nc.tensor.matmul(
                        wps[:, 0:1], wgb[h][:, k * 256:k * 256 + 128],
                        Mb[k][:, 256:257], start=(k == 0), stop=(k == 1))
                p2 = sm.tile([128, 1], f32, tag="su00", name="su00")
                nc.vector.tensor_mul(p2[:], wps[:, 0:1], Mb[0][:, 256:257])
                nc.sync.dma_start(ot_d[0:128, 0:1], p2[:])
                return
            if DBG == 3:
                mm = pcx.tile([128, 512], f32, tag="cx", name="mx")
                nc.tensor.matmul(mm[:, 0:257], xsb[:, 0:128], xsb[:, 0:257],
                                 start=True, stop=True)
                mo = sm.tile([128, 256], f32, tag="mo", name="mo")
                nc.vector.tensor_copy(mo[:], mm[:, 0:256])
                nc.sync.dma_start(ot_d[0:128, 0:256], mo[:])
                return
            if DBG == 2:
                rstds = []
                for h in range(2):
                    r = sm.tile([128, 1], f32, tag=f"h{h}rstd", name=f"h{h}rstd")
                    nc.vector.memset(r[:], 0.057)
                    rstds.append(r)
                run_main(rstds)
                return
            # ---- Gram matrix M = x^T x with ones-column (-> u) ----
            Mb = []
            for half in range(2):
                mm = pcx.tile([128, 512], f32, tag="cx", name="mx")
                for ti in range(NT):
                    o = ti * XSW
                    nc.tensor.matmul(
                        mm[:, 0:257], xsb[:, o + half * 128:o + half * 128 + 128],
                        xsb[:, o:o + 257], start=(ti == 0), stop=(ti == NT - 1))
                mb = sm.tile([128, 257], bf16, tag=f"mb{half}", name=f"mb{half}")
                nc.vector.tensor_copy(mb[:], mm[:, 0:257])
                Mb.append(mb)

            # ---- per-head analytic InstanceNorm stats -> rstd ----
            rstds = []
            for h in range(2):
                sq = sm.tile([128, 2], f32, tag=f"sq{h}", name=f"sq{h}")
                prod = sm.tile([128, 256], f32, tag="prod", name="prod", bufs=2)
                for dti in range(2):
                    dsl = slice(dti * 128, (dti + 1) * 128)
                    dps = pcx.tile([128, 512], f32, tag="cx", name="dps")
                    for k in range(2):
                        nc.tensor.matmul(
                            dps[:, 0:256], wgbt[h][:, k * 256 + dti * 128:
                                                   k * 256 + dti * 128 + 128],
                            Mb[k][:, 0:256], start=(k == 0), stop=(k == 1))
                    db = sm.tile([128, 256], bf16, tag="db", name="db", bufs=2)
                    nc.vector.tensor_copy(db[:], dps[:, 0:256])
                    fps = pcx.tile([128, 512], f32, tag="cx", name="fps")
                    for k in range(2):
                        nc.tensor.matmul(
                            fps[:, 0:256], Mb[k][:, dsl],
                            wgb[h][:, k * 256:k * 256 + 256],
                            start=(k == 0), stop=(k == 1))
                    part = sm.tile([128, 1], f32, tag=f"pt{h}{dti}", name=f"pt{h}{dti}")
                    nc.vector.tensor_mul(prod[:], fps[:, 0:256], db[:])
                    nc.vector.tensor_reduce(part[:], prod[:],
                                            axis=mybir.AxisListType.X, op=ALU.add)
                    if dti == 0:
                        nc.vector.tensor_copy(sq[:, 0:1], part[:])
                    else:
                        nc.vector.tensor_add(sq[:, 0:1], sq[:, 0:1], part[:])
                # sum(scores) = u^T Wg u via wgu = Wg^T u then dot with u
                for dti in range(2):
                    wps = pcx.tile([128, 512], f32, tag="cx", name="wps")
                    for k in range(2):
                        nc.tensor.matmul(
                            wps[:, 0:1], wgb[h][:, k * 256 + dti * 128:
                                                k * 256 + dti * 128 + 128],
                            Mb[k][:, 256:257], start=(k == 0), stop=(k == 1))
                    part = sm.tile([128, 1], f32, tag=f"su{h}{dti}", name=f"su{h}{dti}")
                    nc.vector.tensor_mul(part[:], wps[:, 0:1], Mb[dti][:, 256:257])
                    if dti == 0:
                        nc.vector.tensor_copy(sq[:, 1:2], part[:])
                    else:
                        nc.vector.tensor_add(sq[:, 1:2], sq[:, 1:2], part[:])
                red = sm.tile([128, 2], f32, tag=f"red{h}", name=f"red{h}")
                nc.gpsimd.partition_all_reduce(red[:], sq[:], channels=128,
                                               reduce_op=bass_isa.ReduceOp.add)
                me = sm.tile([128, 2], f32, tag=f"me{h}", name=f"me{h}")
                nc.vector.tensor_scalar_mul(me[:], red[:], 1.0 / CNT)
                mm2 = sm.tile([128, 1], f32, tag=f"mm2{h}", name=f"mm2{h}")
                nc.vector.tensor_mul(mm2[:], me[:, 1:2], me[:, 1:2])
                ve = sm.tile([128, 1], f32, tag=f"ve{h}", name=f"ve{h}")
                nc.vector.scalar_tensor_tensor(
                    out=ve[:], in0=mm2[:], scalar=-1.0, in1=me[:, 0:1],
                    op0=ALU.mult, op1=ALU.add)
                nc.vector.tensor_scalar_add(ve[:], ve[:], EPS)
                rstds.append(rsqrt_dve(ve, f"h{h}"))

            if DBG == 1:
                nc.sync.dma_start(ot_d[0:128, 0:1], rstds[1][:])
                return
            run_main(rstds)

        def run_main(rstds):
            sts = {}   # sts[h][ti] bf16 [128, S]
            vbs = {0: [], 1: []}  # [128, 257] bf16, col 256 = real-t indicator
            ctxT = [cxo.tile([128, 256], f32, tag=f"ct{si}", name=f"ct{si}")
                    for si in range(NS)]

            def emit_g(h):
                """G^T[c2, s] = sum_c1 wg[c1,c2] xt[c1, s]; drain on ACT."""
                g = [gv.tile([128, S], bf16, tag=f"g{d}", name=f"g{d}", bufs=2)
                     for d in range(2)]
                for dti in range(2):
                    wsl = [slice(k * 256 + dti * 128, k * 256 + dti * 128 + 128)
                           for k in range(2)]
                    for lo, hi in CH:
                        pa = psc.tile([128, 512], f32, tag="sc", name="ga")
                        for k in range(2):
                            nc.tensor.matmul(pa[:, 0:hi - lo], wgb[h][:, wsl[k]],
                                             xtb[:, k * SP + lo:k * SP + hi],
                                             start=(k == 0), stop=(k == 1))
                        nc.scalar.activation(out=g[dti][:, lo:hi],
                                             in_=pa[:, 0:hi - lo], func=AF.Copy)
                return g

            def emit_v_tile(h, ti):
                """V'[t, d] chain + ones column; drain on DVE."""
                tsl = slice(ti * 128, (ti + 1) * 128)
                vp = pcx.tile([128, 512], f32, tag="cx", name="vp")
                for k in range(2):
                    nc.tensor.matmul(vp[:, 0:256], xtb[:, k * SP + tsl.start:
                                                       k * SP + tsl.stop],
                                     wvob[h][:, k * 256:k * 256 + 256],
                                     start=(k == 0), stop=(k == 1))
                vb = gv.tile([128, 257], bf16, tag=f"v{ti}", name=f"v{ti}", bufs=2)
                nc.vector.tensor_copy(vb[:, 0:256], vp[:, 0:256])
                # rows >= LAST_T of tile NT-1 are never read (K=32 slice)
                ones_to = LAST_T if ti == NT - 1 else 128
                nc.vector.memset(vb[0:ones_to, 256:257], 1.0)
                vbs[h].append(vb)

            def emit_scores_tile(h, g, ti):
                tsl = slice(ti * 128, (ti + 1) * 128)
                st = sts[h][ti]
                for lo, hi in CH:
                    pa = psc.tile([128, 512], f32, tag="sc", name="sa")
                    for k in range(2):
                        nc.tensor.matmul(pa[:, 0:hi - lo],
                                         xtb[:, k * SP + tsl.start:k * SP + tsl.stop],
                                         g[k][:, lo:hi], start=(k == 0), stop=(k == 1))
                    nc.scalar.activation(out=st[:, lo:hi], in_=pa[:, 0:hi - lo],
                                         func=AF.Exp, scale=rstds[h][:])

            def emit_ctx_tile(h, si):
                ssl = slice(si * 128, min((si + 1) * 128, S))
                rows = ssl.stop - ssl.start
                cp = pcx.tile([128, 512], f32, tag="cx", name="cp")
                for ti in range(NT):
                    kk = LAST_T if ti == NT - 1 else 128
                    nc.tensor.matmul(cp[0:rows, 0:257], sts[h][ti][0:kk, ssl],
                                     vbs[h][ti][0:kk, :],
                                     start=(ti == 0), stop=(ti == NT - 1))
                rec = sm.tile([128, 1], f32, tag=f"rc{si % 2}", name=f"rc{si % 2}",
                              bufs=2)
                nc.vector.reciprocal(rec[0:rows, :], cp[0:rows, 256:257])
                if h == 0:
                    nc.vector.tensor_scalar(
                        out=ctxT[si][0:rows, :], in0=cp[0:rows, 0:256],
                        scalar1=rec[0:rows, :], scalar2=None, op0=ALU.mult)
                else:
                    nc.vector.scalar_tensor_tensor(
                        out=ctxT[si][0:rows, :], in0=cp[0:rows, 0:256],
                        scalar=rec[0:rows, :], in1=ctxT[si][0:rows, :],
                        op0=ALU.mult, op1=ALU.add)
                    nc.sync.dma_start(ot_d[ssl, :], ctxT[si][0:rows, :])

            # phase A: head-0 G/V', scores+exp; V'1 interleaved
            sts[0] = [stp.tile([128, S], bf16, tag=f"st{ti}", name=f"st{ti}", bufs=2)
                      for ti in range(NT)]
            g0 = emit_g(0)
            for ti in range(NT):
                emit_v_tile(0, ti)
            for ti in range(NT):
                emit_scores_tile(0, g0, ti)
                emit_v_tile(1, ti)
            # phase B: head-1 G/scores+exp interleaved with head-0 ctx
            sts[1] = [stp.tile([128, S], bf16, tag=f"st{ti}", name=f"st{ti}", bufs=2)
                      for ti in range(NT)]
            g1 = emit_g(1)
            for i in range(NT):
                emit_scores_tile(1, g1, i)
                emit_ctx_tile(0, i)
            # phase C: head-1 ctx + output
            for si in range(NS):
                emit_ctx_tile(1, si)

        for _ in range(reps):
            body()

    nc.finalize()
    return nc


def _get_nc(reps=1):
    key = ("nc", reps)
    if key not in _CACHE:
        _CACHE[key] = _build_nc(reps)
    return _CACHE[key]


def make_in_maps(emb, Wq, Wk, Wv, Wo):
    import ml_dtypes
    bf16 = ml_dtypes.bfloat16
    emb = np.ascontiguousarray(emb, dtype=np.float32)
    Wq = np.asarray(Wq, np.float64)
    Wk = np.asarray(Wk, np.float64)
    Wv = np.asarray(Wv, np.float64)
    Wo = np.asarray(Wo, np.float64)
    wg = np.einsum("hdc,hde->hce", Wq, Wk).astype(np.float32)
    # fold output projection and the mean over H heads into V'
    wvo = (np.einsum("hdc,ed->hce", Wv, Wo) / H).astype(np.float32)
    in_maps = []
    for core in range(8):
        b, gid = core // 2, core % 2
        x = emb[b].transpose(0, 2, 1).reshape(S, C)          # [S, C]
        xp = np.zeros((SP, C), np.float32)
        xp[:S] = x
        # xtb[p, k*SP + t] = x[t, k*128+p]
        xtb = np.ascontiguousarray(
            xp.T.reshape(2, 128, SP).transpose(1, 0, 2).reshape(128, 2 * SP)
        ).astype(bf16)
        # xsb[p, ti*XSW + c] = x[ti*128+p, c]; col 256 = real-row indicator
        xse = np.zeros((SP, XSW), np.float32)
        xse[:S, :C] = x
        xse[:S, 256] = 1.0
        xsb = np.ascontiguousarray(
            xse.reshape(NT, 128, XSW).transpose(1, 0, 2).reshape(128, NT * XSW)
        ).astype(bf16)
        hs = [2 * gid, 2 * gid + 1]
        # w[h][p, k*256 + c2] = w_full[h][k*128+p, c2]
        wgb = np.ascontiguousarray(
            wg[hs].reshape(2, 2, 128, 256)
            .transpose(0, 2, 1, 3).reshape(2, 128, 2 * 256)).astype(bf16)
        wgt = np.ascontiguousarray(wg[hs].transpose(0, 2, 1))
        wgbt = np.ascontiguousarray(
            wgt.reshape(2, 2, 128, 256)
            .transpose(0, 2, 1, 3).reshape(2, 128, 2 * 256)).astype(bf16)
        wvob = np.ascontiguousarray(
            wvo[hs].reshape(2, 2, 128, 256)
            .transpose(0, 2, 1, 3).reshape(2, 128, 2 * 256)).astype(bf16)
        in_maps.append({"xtb": xtb, "xsb": xsb, "wgb": wgb, "wgbt": wgbt,
                        "wvob": wvob})
    return in_maps


def gather_out(results):
    out = np.empty((B, S, C), np.float32)
    for b in range(B):
        out[b] = results[2 * b]["ot"] + results[2 * b + 1]["ot"]
    return out.reshape(B, T, C, N)


def _get_runner():
    """Cached PJRT executable (run_bass_kernel_spmd re-jits per call)."""
    if "runner" in _CACHE:
        return _CACHE["runner"]
    import jax
    from jax.sharding import Mesh, PartitionSpec
    from jax.experimental.shard_map import shard_map
    from concourse import mybir
    from concourse.bass2jax import (_bass_exec_p, install_neuronx_cc_hook,
                                    partition_id_tensor)

    install_neuronx_cc_hook()
    nc = _get_nc()
    in_names, out_names, out_avals, zero_shapes = [], [], [], []
    partition_name = nc.partition_id_tensor.name if nc.partition_id_tensor else None
    for alloc in nc.m.functions[0].allocations:
        if not isinstance(alloc, mybir.MemoryLocationSet):
            continue
        name = alloc.memorylocations[0].name
        if alloc.kind == "ExternalInput":
            if name != partition_name:
                in_names.append(name)
        elif alloc.kind == "ExternalOutput":
            shape = tuple(alloc.tensor_shape)
            dtype = mybir.dt.np(alloc.dtype)
            out_names.append(name)
            out_avals.append(jax.core.ShapedArray(shape, dtype))
            zero_shapes.append((shape, dtype))
    n_params = len(in_names)
    all_in = list(in_names) + list(out_names)
    if partition_name is not None:
        all_in.append(partition_name)

    def _body(*args):
        operands = list(args)
        if partition_name is not None:
            operands.append(partition_id_tensor())
        return tuple(_bass_exec_p.bind(
            *operands, out_avals=tuple(out_avals), in_names=tuple(all_in),
            out_names=tuple(out_names), lowering_input_output_aliases=(),
            sim_require_finite=True, sim_require_nnan=True, nc=nc))

    n_cores = 8
    mesh = Mesh(np.asarray(jax.devices()[:n_cores]), ("core",))
    sharded = jax.jit(
        shard_map(_body, mesh=mesh,
                  in_specs=(PartitionSpec("core"),) * (n_params + len(out_names)),
                  out_specs=(PartitionSpec("core"),) * len(out_names),
                  check_rep=False),
        keep_unused=True)

    def run(in_maps):
        per_core = [[np.asarray(m[nm]) for nm in in_names] for m in in_maps]
        concat_in = [np.concatenate([per_core[c][i] for c in range(n_cores)], axis=0)
                     for i in range(n_params)]
        concat_zeros = [np.zeros((n_cores * s[0], *s[1:]), d)
                        for (s, d) in zero_shapes]
        outs = sharded(*concat_in, *concat_zeros)
        return [{out_names[i]: np.asarray(outs[i]).reshape(
                     n_cores, *out_avals[i].shape)[c]
                 for i in range(len(out_names))} for c in range(n_cores)]

    _CACHE["runner"] = run
    return run


def kernel(emb, Wq, Wk, Wv, Wo):
    in_maps = make_in_maps(emb, Wq, Wk, Wv, Wo)
    try:
        return gather_out(_get_runner()(in_maps))
    except Exception:
        from concourse.bass_utils import run_bass_kernel_spmd
        nc = _get_nc()
        res = run_bass_kernel_spmd(nc, in_maps, list(range(8)))
        return gather_out(res.results)
